# revision 1
# baseline (speedup 1.0000x reference)
"""DeepSeek-style MoE (32 routed experts, top-8, grouped routing, 2 shared experts)
on 8 Trainium2 NeuronCores via Bass/Tile.

Strategy (expert-parallel, load-balanced):
- Host computes the routing (sigmoid gate + grouped top-k, bit-matching the
  reference via jax-on-CPU) and gathers each expert's tokens.
- Experts with more than SPLIT_Q tokens are split into balanced virtual
  pieces (a small greedy search reassigns splits to minimize total static
  capacity); pieces are sorted by size, piece of rank r goes to core r%8,
  slot r//8. All cores run the SAME program: n_slots slots with static
  capacities equal to the per-slot max piece size (rounded up to 8 — walrus
  rejects odd fp32r matmul widths), so the kernel is SPMD-static while the
  work stays balanced across cores (sum of caps 4536 vs the 4096 ideal for
  the seed-0 routing).
- Per slot the device computes the expert MLP on transposed activations
  entirely with fp32r matmuls (full PE rate at moving dim >= 256, ~1e-3 max
  rel err):
      gu^T = w_gate_up^T @ x^T               (PSUM, 16 K-chunks over H)
      h    = silu(gu_gate) * gu_up           (ACT silu + DVE mul, fp32r SBUF)
      y^T  = w_down^T @ h                    (PSUM -> ACT copy -> DRAM)
  Inputs (xg/wgu) stream on the SP HWDGE queue, phase-B weights and outputs
  on the ACT HWDGE queue, in small per-chunk DMAs, so the serial DMA fabric
  interleaves streams and the next slot's prefetch is never head-of-line
  blocked.
- The shared MLP runs 2-way tensor-parallel (intermediate dim) x 4-way
  data-parallel (tokens).
- Host combines: out = sum of shared partials + scatter-add of
  2.5 * topk_w * y rows per expert piece.

Self-contained: only numpy/jax/concourse imports, shapes hardcoded.
"""
import numpy as np

import concourse.bass as bass
import concourse.mybir as mybir
import concourse.tile as tile
from concourse.bass_utils import run_bass_kernel_spmd

F32 = mybir.dt.float32
F32R = mybir.dt.float32r

T, H, E, I = 4096, 2048, 32, 1024
TOP_K, N_GROUP, TOPK_GROUP = 8, 8, 4
SI = 2048
ROUTED_SCALING = 2.5
N_CORES = 8
SPLIT_Q = 1152   # experts with more tokens are split into virtual experts
MAX_CHUNK = 1152
# shared expert: SHARED_TP-way split of the intermediate dim x SHARED_DP-way
# split of the tokens (SHARED_TP * SHARED_DP == N_CORES)
SHARED_TP = 2
SHARED_DP = 4
S_TOK = T // SHARED_DP      # tokens per core for the shared MLP
S_SI = SI // SHARED_TP      # intermediate slice per core

_HHC = H // 128   # 16 k-chunks over H
_IC = I // 128    # 8 chunks over I
_SIC = SI // 128  # 16 chunks over SI


# ---------------------------------------------------------------- host routing
def _grouped_topk_host(hidden_states, gate_w, gate_bias):
    """Bit-match the reference's jax fp32 routing, on the CPU backend."""
    import jax

    try:
        jax.config.update("jax_platforms", "axon,cpu")
    except Exception:
        pass
    import jax.numpy as jnp

    cpu = jax.devices("cpu")[0]
    with jax.default_device(cpu):
        hs = jnp.asarray(hidden_states)
        gw = jnp.asarray(gate_w)
        bias = jnp.asarray(gate_bias)
        router_logits = hs @ gw
        scores = jax.nn.sigmoid(router_logits)
        sc = scores + bias[None, :]
        t = sc.shape[0]
        g = sc.reshape(t, N_GROUP, E // N_GROUP)
        group_scores = jax.lax.top_k(g, 2)[0].sum(-1)
        grp_idx = jax.lax.top_k(group_scores, TOPK_GROUP)[1]
        grp_mask = jnp.zeros((t, N_GROUP), sc.dtype).at[
            jnp.arange(t)[:, None], grp_idx].set(1.0)
        tok_mask = jnp.repeat(grp_mask, E // N_GROUP, axis=1)
        masked = jnp.where(tok_mask > 0, sc, -jnp.inf)
        topk_ids = jax.lax.top_k(masked, TOP_K)[1]
        w = jnp.take_along_axis(scores, topk_ids, axis=1)
        w = w / (w.sum(-1, keepdims=True) + 1e-20)
        return np.asarray(w), np.asarray(topk_ids)


def _roundup(x, m):
    return -(-x // m) * m


def _chunk_sizes(cap):
    """Split cap (multiple of 128, >=256) into chunks <= MAX_CHUNK, each a
    multiple of 128 and >= 256."""
    out = []
    rem = cap
    while rem > MAX_CHUNK:
        take = MAX_CHUNK if rem - MAX_CHUNK >= 256 else MAX_CHUNK - 256
        out.append(take)
        rem -= take
    out.append(rem)
    return out


def _n_tiles(chunk):
    """Split chunk (any int >= 256) into matmul N-tiles in [256, 512]
    (fp32r full rate needs a moving dim >= 256)."""
    out = []
    rem = chunk
    while rem > 512:
        take = 512 if rem - 512 >= 256 else rem - 256
        take -= take % 8
        out.append(take)
        rem -= take
    assert 256 <= rem <= 512, rem
    out.append(rem)
    return out


# ---------------------------------------------------------------- bass program
def _build_nc(caps, include_routed=True, include_shared=True):
    nc = bass.Bass()
    CT = sum(caps)
    n_slots = len(caps)

    xg_d = nc.dram_tensor("xg", [H, CT], F32R, kind="ExternalInput")
    wgu_d = nc.dram_tensor("wgu", [n_slots, I // 128, H, 256], F32R, kind="ExternalInput")
    wdn_d = nc.dram_tensor("wdn", [n_slots, H // 128, I, 128], F32R, kind="ExternalInput")
    sgu_d = nc.dram_tensor("sgu", [S_SI // 128, H, 256], F32R, kind="ExternalInput")
    sdn_d = nc.dram_tensor("sdn", [H // 128, S_SI, 128], F32R, kind="ExternalInput")
    xts_d = nc.dram_tensor("xts", [H, S_TOK], F32R, kind="ExternalInput")
    y_d = nc.dram_tensor("y", [H, CT], F32, kind="ExternalOutput")
    ys_d = nc.dram_tensor("ys", [H, S_TOK], F32, kind="ExternalOutput")

    xg_v = xg_d.rearrange("(k p) n -> p k n", p=128)
    xts_v = xts_d.rearrange("(k p) n -> p k n", p=128)
    y_v = y_d.rearrange("(m p) n -> m p n", p=128)
    ys_v = ys_d.rearrange("(m p) n -> m p n", p=128)

    silu = mybir.ActivationFunctionType.Silu
    copy_fn = mybir.ActivationFunctionType.Copy

    from contextlib import ExitStack

    with tile.TileContext(nc) as tc, ExitStack() as ctx:
        xg_pool = ctx.enter_context(tc.tile_pool(name="xgp", bufs=1))
        w_pool = ctx.enter_context(tc.tile_pool(name="wp", bufs=2))
        dn_pool = ctx.enter_context(tc.tile_pool(name="dnp", bufs=3))
        h_pool = ctx.enter_context(tc.tile_pool(name="hp", bufs=2))
        y_pool = ctx.enter_context(tc.tile_pool(name="yp", bufs=2))
        psA = ctx.enter_context(tc.tile_pool(name="psA", bufs=3, space="PSUM"))
        psB = ctx.enter_context(tc.tile_pool(name="psB", bufs=2, space="PSUM"))

        first_chunk = [True]

        def mlp_chunk(gu_pair_srcs, dn_m_srcs, x_src, y_dst, cw, n_pairs, n_kA):
            """One token-chunk of one expert MLP.

            gu_pair_srcs[mp]: DRAM AP [p, n_kA, 256] (gate|up cols of pair mp)
            dn_m_srcs[m]:     DRAM AP [p, n_pairs, 128]
            x_src:            DRAM AP [p, n_kA, cw]
            y_dst[m]:         DRAM AP [p, cw]
            """
            tiles = _n_tiles(cw)

            def wp_load(mp):
                wp = w_pool.tile([128, n_kA, 256], F32R, tag="w", name="wpt")
                qk = n_kA // 8
                for q in range(8):
                    nc.sync.dma_start(out=wp[:, q * qk:(q + 1) * qk],
                                      in_=gu_pair_srcs[mp][:, q * qk:(q + 1) * qk])
                return wp

            # per-k-chunk xg DMAs: keeps single transfers small so the serial
            # DMA fabric interleaves them with the previous slot's dn/y stream.
            # For the program's first chunk, issue only the critical quarter of
            # wgu pair 0 and xg chunk 0 first so the first matmul starts ASAP.
            if first_chunk[0]:
                first_chunk[0] = False
                wp_next = w_pool.tile([128, n_kA, 256], F32R, tag="w", name="wpt")
                qk = n_kA // 4
                nc.sync.dma_start(out=wp_next[:, :qk], in_=gu_pair_srcs[0][:, :qk])
                xt = xg_pool.tile([128, n_kA, cw], F32R, tag="xg", name="xt")
                nc.sync.dma_start(out=xt[:, 0], in_=x_src[:, 0])
                for q in range(1, 4):
                    nc.sync.dma_start(out=wp_next[:, q * qk:(q + 1) * qk],
                                      in_=gu_pair_srcs[0][:, q * qk:(q + 1) * qk])
                for k in range(1, n_kA):
                    nc.sync.dma_start(out=xt[:, k], in_=x_src[:, k])
            else:
                wp_next = wp_load(0)
                xt = xg_pool.tile([128, n_kA, cw], F32R, tag="xg", name="xt")
                for k in range(n_kA):
                    nc.sync.dma_start(out=xt[:, k], in_=x_src[:, k])
            ht = h_pool.tile([128, n_pairs, cw], F32R, tag="h", name="ht")
            for mp in range(n_pairs):
                wp = wp_next
                if mp + 1 < n_pairs:
                    wp_next = wp_load(mp + 1)
                off = 0
                for nt in tiles:
                    g = psA.tile([128, 512], F32, tag="g", name="gps", bufs=4)[:, :nt]
                    u = psA.tile([128, 512], F32, tag="u", name="ups", bufs=2)[:, :nt]
                    for k in range(n_kA):
                        nc.tensor.matmul(
                            g, wp[:, k, 0:128], xt[:, k, off:off + nt],
                            start=(k == 0), stop=(k == n_kA - 1))
                        nc.tensor.matmul(
                            u, wp[:, k, 128:256], xt[:, k, off:off + nt],
                            start=(k == 0), stop=(k == n_kA - 1))
                    hslice = ht[:, mp, off:off + nt]
                    nc.scalar.activation(hslice, g, silu)
                    nc.vector.tensor_mul(hslice, hslice, u)
                    off += nt
            # phase B weights and outputs ride the ACT HWDGE queue so the SP
            # queue stays free for the next slot's xg/wgu prefetch; dn loads
            # are emitted one m'-tile ahead so y stores never block them
            def dn_load(m):
                dt_ = dn_pool.tile([128, n_pairs, 128], F32R, tag="dn", name="dnt")
                nc.scalar.dma_start(out=dt_[:], in_=dn_m_srcs[m])
                return dt_
            dn_q = [dn_load(0), dn_load(1)]
            for m in range(_HHC):
                dt_ = dn_q.pop(0)
                if m + 2 < _HHC:
                    dn_q.append(dn_load(m + 2))
                yt = y_pool.tile([128, cw], F32, tag="y", name="yt")
                off = 0
                for nt in tiles:
                    py = psB.tile([128, 512], F32, tag="py", name="pyps")[:, :nt]
                    for k in range(n_pairs):
                        nc.tensor.matmul(
                            py, dt_[:, k, :], ht[:, k, off:off + nt],
                            start=(k == 0), stop=(k == n_pairs - 1))
                    nc.scalar.activation(yt[:, off:off + nt], py, copy_fn)
                    off += nt
                nc.scalar.dma_start(out=y_dst[m], in_=yt[:])

        # shared expert: TP slice of the intermediate dim x DP slice of tokens
        sgu_srcs = [sgu_d[mp].rearrange("(k p) c -> p k c", p=128)
                    for mp in range(S_SI // 128)]
        sdn_srcs = [sdn_d[m].rearrange("(k p) c -> p k c", p=128)
                    for m in range(_HHC)]

        def shared_chunk(t0, t1):
            mlp_chunk(
                sgu_srcs,
                sdn_srcs,
                xts_v[:, :, t0:t1],
                [ys_v[m][:, t0:t1] for m in range(_HHC)],
                t1 - t0, n_pairs=S_SI // 128, n_kA=_HHC,
            )

        # routed slots, with a shared-expert chunk interleaved mid-way to give
        # the scheduler independent PE work across slot junctions
        off = 0
        for s in range(n_slots if include_routed else 0):
            cap = caps[s]
            gu_srcs = [wgu_d[s, mp].rearrange("(k p) c -> p k c", p=128)
                       for mp in range(_IC)]
            for cw in _chunk_sizes(cap):
                dn_srcs = [wdn_d[s, m].rearrange("(k p) c -> p k c", p=128)
                           for m in range(_HHC)]
                o = off
                mlp_chunk(
                    gu_srcs,
                    dn_srcs,
                    xg_v[:, :, o:o + cw],
                    [y_v[m][:, o:o + cw] for m in range(_HHC)],
                    cw, n_pairs=_IC, n_kA=_HHC,
                )
                off += cw
        if include_shared:
            shared_chunk(0, S_TOK)

    _split_wide_waits(nc)
    return nc


# ------------------------------------------------------- walrus wait-limit fix
def _split_wide_waits(nc):
    """walrus codegen allows only 1 sync wait on fused 4-byte matmuls (and few
    on ctrl ops). Hoist extra waits into single-wait NoOps on the same engine."""
    n = 0
    for f in nc.m.functions:
        for bb in f.blocks:
            il = bb.instructions
            i = 0
            while i < len(il):
                inst = il[i]
                si = inst.sync_info
                waits = list(si.on_wait) if si and si.on_wait else []
                cap = 1
                if len(waits) > cap:
                    inst.sync_info = mybir.SyncInfo(
                        on_wait=waits[:cap], on_update=list(si.on_update or []))
                    nops = [
                        mybir.InstNoOp(
                            name=nc.get_next_instruction_name(),
                            sync_info=mybir.SyncInfo(on_wait=[w], on_update=[]),
                            bass_nofuse=True,
                            engine=inst.engine,
                        )
                        for w in waits[cap:]
                    ]
                    il[i:i] = nops
                    i += len(nops)
                    n += len(nops)
                i += 1
    return n


# ------------------------------------------------------------------- assembly
def plan(topk_w, topk_ids):
    """Work assignment: split big experts into virtual pieces (<= SPLIT_Q
    tokens), sort pieces by size, piece of rank r -> core r % 8, slot r // 8.
    Slot capacities are the per-slot maxima; slots are ordered smallest-first.
    Returns (caps, assign, tok_of) where assign[s][c] = (expert, start, n)."""
    counts = np.bincount(topk_ids.ravel(), minlength=E)
    tok_of = [np.nonzero(topk_ids == e) for e in range(E)]
    live = [e for e in range(E) if counts[e] > 0]
    cs = [int(counts[e]) for e in live]

    def split_sizes(c, k):
        return [c // k + (1 if i < c % k else 0) for i in range(k)]

    def sum_caps(ks):
        sizes = sorted((s for c, k in zip(cs, ks) for s in split_sizes(c, k)),
                       reverse=True)
        ns = -(-len(sizes) // N_CORES)
        return sum(max(256, _roundup(sizes[N_CORES * s], 8)) for s in range(ns))

    # start: balanced split of everything above SPLIT_Q, then greedily move
    # splits between experts while it lowers the total static capacity
    ks = [-(-c // SPLIT_Q) for c in cs]
    budget = _roundup(sum(ks), N_CORES) // 1  # piece budget: fill current slots
    best = sum_caps(ks)
    improved = True
    while improved:
        improved = False
        for a in range(len(cs)):
            if sum(ks) < budget:
                ks[a] += 1
                v = sum_caps(ks)
                if v < best:
                    best, improved = v, True
                    continue
                ks[a] -= 1
            for b in range(len(cs)):
                if b == a or ks[b] < 2:
                    continue
                if -(-cs[b] // (ks[b] - 1)) > SPLIT_Q:
                    continue  # keep every piece (and so every cap) <= one chunk
                ks[a] += 1
                ks[b] -= 1
                v = sum_caps(ks)
                if v < best:
                    best, improved = v, True
                    break
                ks[a] -= 1
                ks[b] += 1

    pieces = []
    for e, c, k in zip(live, cs, ks):
        st = 0
        for n in split_sizes(c, k):
            pieces.append((e, st, n))
            st += n
    pieces.sort(key=lambda p: -p[2])
    n_slots = -(-len(pieces) // N_CORES)
    pieces += [(0, 0, 0)] * (n_slots * N_CORES - len(pieces))
    slots = [pieces[N_CORES * s:N_CORES * (s + 1)] for s in range(n_slots)]
    slots.sort(key=lambda sl: sl[0][2])  # ascending cap
    # near-exact per-slot maxima; multiple of 8 for matmul/DMA alignment
    caps = [max(256, _roundup(sl[0][2], 8)) for sl in slots]
    return caps, slots, tok_of


def kernel(hidden_states, gate_w, gate_bias, w_gate_up, w_down,
           shared_gate_up, shared_down):
    hs = np.ascontiguousarray(hidden_states, dtype=np.float32)
    topk_w, topk_ids = _grouped_topk_host(hs, gate_w, gate_bias)
    caps, slots, tok_of = plan(topk_w, topk_ids)
    n_slots = len(caps)
    CT = sum(caps)
    offs = np.concatenate([[0], np.cumsum(caps)])[:n_slots]

    w_gate_up = np.asarray(w_gate_up, dtype=np.float32)
    w_down = np.asarray(w_down, dtype=np.float32)
    shared_gate_up = np.asarray(shared_gate_up, dtype=np.float32)
    shared_down = np.asarray(shared_down, dtype=np.float32)

    # shared tensors: per TP-slice of the intermediate dim
    SGU_tp, SDN_tp = [], []
    for tp in range(SHARED_TP):
        base = tp * S_SI
        sgu = np.empty((S_SI // 128, H, 256), np.float32)
        for mp in range(S_SI // 128):
            sgu[mp, :, 0:128] = shared_gate_up[:, base + mp * 128: base + (mp + 1) * 128]
            sgu[mp, :, 128:256] = shared_gate_up[
                :, SI + base + mp * 128: SI + base + (mp + 1) * 128]
        SGU_tp.append(sgu)
        sdn = np.empty((H // 128, S_SI, 128), np.float32)
        for m in range(H // 128):
            sdn[m] = shared_down[base:base + S_SI, m * 128:(m + 1) * 128]
        SDN_tp.append(sdn)
    XTS_dp = [
        np.ascontiguousarray(hs[dp * S_TOK:(dp + 1) * S_TOK].T)
        for dp in range(SHARED_DP)
    ]

    in_maps = []
    for c in range(N_CORES):
        XG = np.zeros((H, CT), np.float32)
        WGU = np.zeros((n_slots, I // 128, H, 256), np.float32)
        WDN = np.zeros((n_slots, H // 128, I, 128), np.float32)
        for s in range(n_slots):
            e, st, n = slots[s][c]
            if n == 0:
                continue
            idx = tok_of[e][0][st:st + n]
            XG[:, offs[s]:offs[s] + n] = hs[idx].T
            wg = w_gate_up[e]
            for mp in range(I // 128):
                WGU[s, mp, :, 0:128] = wg[:, mp * 128:(mp + 1) * 128]
                WGU[s, mp, :, 128:256] = wg[:, I + mp * 128: I + (mp + 1) * 128]
            wd = w_down[e]
            for m in range(H // 128):
                WDN[s, m] = wd[:, m * 128:(m + 1) * 128]
        tp, dp = c // SHARED_DP, c % SHARED_DP
        in_maps.append({
            "xg": XG, "wgu": WGU, "wdn": WDN,
            "sgu": SGU_tp[tp], "sdn": SDN_tp[tp], "xts": XTS_dp[dp],
        })

    nc = _build_nc(caps)
    res = run_bass_kernel_spmd(nc, in_maps, list(range(N_CORES)))

    out = np.zeros((T, H), np.float32)
    for c in range(N_CORES):
        dp = c % SHARED_DP
        out[dp * S_TOK:(dp + 1) * S_TOK] += res.results[c]["ys"].T
    for c in range(N_CORES):
        y = res.results[c]["y"]
        for s in range(n_slots):
            e, st, n = slots[s][c]
            if n == 0:
                continue
            idx = tok_of[e][0][st:st + n]
            kpos = tok_of[e][1][st:st + n]
            wts = topk_w[idx, kpos].astype(np.float32) * ROUTED_SCALING
            out[idx] += wts[:, None] * y[:, offs[s]:offs[s] + n].T
    return out



# revision 3
# speedup vs baseline: 1.0045x; 1.0045x over previous
"""DeepSeek-style MoE (32 routed experts, top-8, grouped routing, 2 shared experts)
on 8 Trainium2 NeuronCores via Bass/Tile.

Strategy (expert-parallel, fp8 DoubleRow matmuls with hi/lo error compensation):
- Host computes the routing (sigmoid gate + grouped top-k, bit-matching the
  reference via jax-on-CPU) and gathers each expert's tokens.
- Experts are split into balanced virtual pieces; pieces sorted by size,
  piece of rank r goes to core r%8, slot r//8. All cores run the SAME
  program (slot capacities = per-slot max piece size), so the kernel is
  SPMD-static while the work stays balanced.
- All matmuls run as fp8e4 (e4m3) DoubleRow instructions (2 k-tiles of 128
  per instruction, 0.5 cycles/row = 4x the bf16 FLOP rate). Every operand
  O is decomposed on the host (or on-chip for h) into hi+lo e4m3 parts:
  O/s = Oh + Ol with Ol the quantized residual, so O is represented to
  ~6e-4 relative. A matmul W@x is computed with three terms
      Wh@xh + Wl@xh + Wh@xl      (dropping the ~7e-7 Wl@xl term)
  at 0.75x the bf16-equivalent PE time. k-chunks are paired into the two
  DoubleRow k-tiles, so all operands keep their natural chunk-major layout.
- Per expert-slot MLP on transposed activations:
      gu^T = w_gate_up^T @ x^T     (3-term DR into PSUM, 8 chunk-pairs)
      t    = silu(g * sA)          (ACT, f32)
      hb   = (u * sA*CH) * t       (DVE scalar_tensor_tensor, f32, h*CH)
      hh   = fp8(hb)               (Pool copy)
      hl   = fp8(hb - hh)          (DVE scalar_tensor_tensor)
      y^T  = w_down^T @ h          (3-term DR, PSUM -> ACT copy*sWd/CH -> bf16)
- The shared MLP runs 2-way tensor-parallel (intermediate dim) x 4-way
  data-parallel (tokens), same scheme.
- Host combines in fp32: out = shared partials + 2.5 * topk_w * y rows.

Self-contained: only numpy/jax/ml_dtypes/concourse imports, shapes hardcoded.
"""
import numpy as np
import ml_dtypes

import concourse.bass as bass
import concourse.mybir as mybir
import concourse.tile as tile
from concourse.bass_utils import run_bass_kernel_spmd

F32 = mybir.dt.float32
BF16 = mybir.dt.bfloat16
F8 = mybir.dt.float8e4
E4 = ml_dtypes.float8_e4m3
BF = ml_dtypes.bfloat16
DR = mybir.MatmulPerfMode.DoubleRow
MULT = mybir.AluOpType.mult
ADD = mybir.AluOpType.add

T, H, E, I = 4096, 2048, 32, 1024
TOP_K, N_GROUP, TOPK_GROUP = 8, 8, 4
SI = 2048
ROUTED_SCALING = 2.5
N_CORES = 8
SPLIT_Q = 1152   # experts with more tokens are split into virtual experts
MAX_CHUNK = 1152
# shared expert: SHARED_TP-way split of the intermediate dim x SHARED_DP-way
# split of the tokens (SHARED_TP * SHARED_DP == N_CORES)
SHARED_TP = 2
SHARED_DP = 4
S_TOK = T // SHARED_DP      # tokens per core for the shared MLP
S_SI = SI // SHARED_TP      # intermediate slice per core

_HHC = H // 128   # 16 k-chunks over H
_IC = I // 128    # 8 pairs over I (also h k-chunks)
_SIC = S_SI // 128  # 8 shared pairs

QMAX = 224.0      # e4m3 target max (true max 240; margin for rounding)
CH = 8.0          # h -> fp8 scale (h*CH must stay below ~240)


# ---------------------------------------------------------------- host routing
def _grouped_topk_host(hidden_states, gate_w, gate_bias):
    """Bit-match the reference's jax fp32 routing, on the CPU backend."""
    import jax

    try:
        jax.config.update("jax_platforms", "axon,cpu")
    except Exception:
        pass
    import jax.numpy as jnp

    cpu = jax.devices("cpu")[0]
    with jax.default_device(cpu):
        hs = jnp.asarray(hidden_states)
        gw = jnp.asarray(gate_w)
        bias = jnp.asarray(gate_bias)
        router_logits = hs @ gw
        scores = jax.nn.sigmoid(router_logits)
        sc = scores + bias[None, :]
        t = sc.shape[0]
        g = sc.reshape(t, N_GROUP, E // N_GROUP)
        group_scores = jax.lax.top_k(g, 2)[0].sum(-1)
        grp_idx = jax.lax.top_k(group_scores, TOPK_GROUP)[1]
        grp_mask = jnp.zeros((t, N_GROUP), sc.dtype).at[
            jnp.arange(t)[:, None], grp_idx].set(1.0)
        tok_mask = jnp.repeat(grp_mask, E // N_GROUP, axis=1)
        masked = jnp.where(tok_mask > 0, sc, -jnp.inf)
        topk_ids = jax.lax.top_k(masked, TOP_K)[1]
        w = jnp.take_along_axis(scores, topk_ids, axis=1)
        w = w / (w.sum(-1, keepdims=True) + 1e-20)
        return np.asarray(w), np.asarray(topk_ids)


def _roundup(x, m):
    return -(-x // m) * m


def _chunk_sizes(cap):
    """Split cap (multiple of 8) into chunks <= MAX_CHUNK, multiple of 8."""
    out = []
    rem = cap
    while rem > MAX_CHUNK:
        take = MAX_CHUNK
        out.append(take)
        rem -= take
    out.append(rem)
    return out


def _n_tiles(chunk):
    """Split chunk into matmul N-tiles <= 512 (DR moving dim = 2*nt <= 1024)."""
    out = []
    rem = chunk
    while rem > 512:
        out.append(512)
        rem -= 512
    out.append(rem)
    return out


# ---------------------------------------------------------------- bass program
def _build_nc(caps, sA_r=1.0, sB_r=1.0, sA_s=1.0, sB_s=1.0,
              include_routed=True, include_shared=True):
    """sA_r: silu input scale for routed (sWgu*sX); sB_r: y scale (sWdn/CH);
    sA_s/sB_s: same for the shared MLP."""
    nc = bass.Bass()
    CT = sum(caps)
    n_slots = len(caps)

    xgh_d = nc.dram_tensor("xgh", [128, _HHC, CT], F8, kind="ExternalInput")
    xgl_d = nc.dram_tensor("xgl", [128, _HHC, CT], F8, kind="ExternalInput")
    wguh_d = nc.dram_tensor("wguh", [n_slots, _IC, 128, _HHC, 256], F8, kind="ExternalInput")
    wgul_d = nc.dram_tensor("wgul", [n_slots, _IC, 128, _HHC, 256], F8, kind="ExternalInput")
    wdnh_d = nc.dram_tensor("wdnh", [n_slots, _HHC, 128, _IC, 128], F8, kind="ExternalInput")
    wdnl_d = nc.dram_tensor("wdnl", [n_slots, _HHC, 128, _IC, 128], F8, kind="ExternalInput")
    sguh_d = nc.dram_tensor("sguh", [_SIC, 128, _HHC, 256], F8, kind="ExternalInput")
    sgul_d = nc.dram_tensor("sgul", [_SIC, 128, _HHC, 256], F8, kind="ExternalInput")
    sdnh_d = nc.dram_tensor("sdnh", [_HHC, 128, _SIC, 128], F8, kind="ExternalInput")
    sdnl_d = nc.dram_tensor("sdnl", [_HHC, 128, _SIC, 128], F8, kind="ExternalInput")
    xtsh_d = nc.dram_tensor("xtsh", [128, _HHC, S_TOK], F8, kind="ExternalInput")
    xtsl_d = nc.dram_tensor("xtsl", [128, _HHC, S_TOK], F8, kind="ExternalInput")
    y_d = nc.dram_tensor("y", [_HHC, 128, CT], BF16, kind="ExternalOutput")
    ys_d = nc.dram_tensor("ys", [_HHC, 128, S_TOK], BF16, kind="ExternalOutput")

    silu = mybir.ActivationFunctionType.Silu
    copy_fn = mybir.ActivationFunctionType.Copy

    from contextlib import ExitStack

    with tile.TileContext(nc) as tc, ExitStack() as ctx:
        xg_pool = ctx.enter_context(tc.tile_pool(name="xgp", bufs=2))
        w_pool = ctx.enter_context(tc.tile_pool(name="wp", bufs=2))
        dn_pool = ctx.enter_context(tc.tile_pool(name="dnp", bufs=3))
        h_pool = ctx.enter_context(tc.tile_pool(name="hp", bufs=2))
        t_pool = ctx.enter_context(tc.tile_pool(name="tp", bufs=2))
        y_pool = ctx.enter_context(tc.tile_pool(name="yp", bufs=2))
        psA = ctx.enter_context(tc.tile_pool(name="psA", bufs=2, space="PSUM"))
        psB = ctx.enter_context(tc.tile_pool(name="psB", bufs=2, space="PSUM"))

        first_chunk = [True]

        def mlp_chunk(gu_srcs, dn_srcs, x_srcs, y_dst, cw, n_pairs, n_kA,
                      sA, sB):
            """One token-chunk of one expert MLP (all fp8 hi/lo DoubleRow).

            gu_srcs[hl][mp]: DRAM AP [128, n_kA, 256] (gate|up cols of pair mp)
            dn_srcs[hl][m]:  DRAM AP [128, n_pairs, 128]
            x_srcs[hl]:      DRAM AP [128, n_kA, cw]
            y_dst[m]:        DRAM AP [128, cw] (bf16)
            """
            tiles = _n_tiles(cw)
            ncpA = n_kA // 2
            ncpB = n_pairs // 2

            def wp_load(mp):
                wph = w_pool.tile([128, n_kA, 256], F8, tag="wh", name="wph")
                wpl = w_pool.tile([128, n_kA, 256], F8, tag="wl", name="wpl")
                qk = n_kA // 4
                for q in range(4):
                    s = slice(q * qk, (q + 1) * qk)
                    nc.sync.dma_start(out=wph[:, s], in_=gu_srcs[0][mp][:, s])
                    nc.sync.dma_start(out=wpl[:, s], in_=gu_srcs[1][mp][:, s])
                return wph, wpl

            # per-k-chunk xg DMAs keep transfers small so the DMA fabric can
            # interleave them with the previous slot's dn/y stream. For the
            # program's first chunk, issue only the first quarter of wgu pair 0
            # and xh chunk 0 first so the first matmul starts ASAP.
            if first_chunk[0]:
                first_chunk[0] = False
                wph = w_pool.tile([128, n_kA, 256], F8, tag="wh", name="wph")
                wpl = w_pool.tile([128, n_kA, 256], F8, tag="wl", name="wpl")
                qk = n_kA // 4
                nc.sync.dma_start(out=wph[:, :qk], in_=gu_srcs[0][0][:, :qk])
                xh = xg_pool.tile([128, n_kA, cw], F8, tag="xh", name="xht")
                xl = xg_pool.tile([128, n_kA, cw], F8, tag="xl", name="xlt")
                nc.sync.dma_start(out=xh[:, 0], in_=x_srcs[0][:, 0])
                nc.sync.dma_start(out=wpl[:, :qk], in_=gu_srcs[1][0][:, :qk])
                for q in range(1, 4):
                    s = slice(q * qk, (q + 1) * qk)
                    nc.sync.dma_start(out=wph[:, s], in_=gu_srcs[0][0][:, s])
                    nc.sync.dma_start(out=wpl[:, s], in_=gu_srcs[1][0][:, s])
                for k in range(1, n_kA):
                    nc.sync.dma_start(out=xh[:, k], in_=x_srcs[0][:, k])
                for k in range(n_kA):
                    nc.sync.dma_start(out=xl[:, k], in_=x_srcs[1][:, k])
                wp_next = (wph, wpl)
            else:
                wp_next = wp_load(0)
                xh = xg_pool.tile([128, n_kA, cw], F8, tag="xh", name="xht")
                xl = xg_pool.tile([128, n_kA, cw], F8, tag="xl", name="xlt")
                for k in range(n_kA):
                    nc.sync.dma_start(out=xh[:, k], in_=x_srcs[0][:, k])
                    nc.sync.dma_start(out=xl[:, k], in_=x_srcs[1][:, k])

            hh = h_pool.tile([128, n_pairs, cw], F8, tag="hh", name="hht")
            hl = h_pool.tile([128, n_pairs, cw], F8, tag="hl", name="hlt")
            for mp in range(n_pairs):
                wph, wpl = wp_next
                if mp + 1 < n_pairs:
                    wp_next = wp_load(mp + 1)
                off = 0
                for nt in tiles:
                    g = psA.tile([128, 512], F32, tag="g", name="gps")[:, :nt]
                    u = psA.tile([128, 512], F32, tag="u", name="ups")[:, :nt]
                    for cp in range(ncpA):
                        sl = slice(2 * cp, 2 * cp + 2)
                        xhs = xh[:, sl, off:off + nt]
                        xls = xl[:, sl, off:off + nt]
                        st = cp == 0
                        sp = cp == ncpA - 1
                        for dst, c0 in ((g, 0), (u, 128)):
                            ws_h = wph[:, sl, c0:c0 + 128]
                            ws_l = wpl[:, sl, c0:c0 + 128]
                            nc.tensor.matmul(dst, ws_h, xhs, start=st,
                                             stop=False, perf_mode=DR)
                            nc.tensor.matmul(dst, ws_l, xhs, start=False,
                                             stop=False, perf_mode=DR)
                            nc.tensor.matmul(dst, ws_h, xls, start=False,
                                             stop=sp, perf_mode=DR)
                    t = t_pool.tile([128, 512], F32, tag="t", name="tt")[:, :nt]
                    hb = t_pool.tile([128, 512], F32, tag="hb", name="hbt")[:, :nt]
                    nc.scalar.activation(t, g, silu, scale=sA)
                    # hb = (u * sA*CH) * t = h * CH
                    nc.vector.scalar_tensor_tensor(hb, u, sA * CH, t, MULT, MULT)
                    hhs = hh[:, mp, off:off + nt]
                    hls = hl[:, mp, off:off + nt]
                    nc.gpsimd.tensor_copy(hhs, hb)
                    # hl = hb - hh (quantized residual)
                    nc.vector.scalar_tensor_tensor(hls, hhs, -1.0, hb, MULT, ADD)
                    off += nt
            # phase B weights and outputs ride the ACT HWDGE queue so the SP
            # queue stays free for the next slot's prefetch; dn loads are
            # emitted one m-tile ahead so y stores never block them
            def dn_load(m):
                dh = dn_pool.tile([128, n_pairs, 128], F8, tag="dh", name="dht")
                dl = dn_pool.tile([128, n_pairs, 128], F8, tag="dl", name="dlt")
                nc.scalar.dma_start(out=dh[:], in_=dn_srcs[0][m])
                nc.scalar.dma_start(out=dl[:], in_=dn_srcs[1][m])
                return dh, dl
            dn_q = [dn_load(0), dn_load(1)]
            for m in range(_HHC):
                dh, dl = dn_q.pop(0)
                if m + 2 < _HHC:
                    dn_q.append(dn_load(m + 2))
                yt = y_pool.tile([128, cw], BF16, tag="y", name="yt")
                off = 0
                for nt in tiles:
                    py = psB.tile([128, 512], F32, tag="py", name="pyps")[:, :nt]
                    for cp in range(ncpB):
                        sl = slice(2 * cp, 2 * cp + 2)
                        hhs = hh[:, sl, off:off + nt]
                        hls = hl[:, sl, off:off + nt]
                        nc.tensor.matmul(py, dh[:, sl, :], hhs, start=(cp == 0),
                                         stop=False, perf_mode=DR)
                        nc.tensor.matmul(py, dl[:, sl, :], hhs, start=False,
                                         stop=False, perf_mode=DR)
                        nc.tensor.matmul(py, dh[:, sl, :], hls, start=False,
                                         stop=(cp == ncpB - 1), perf_mode=DR)
                    nc.scalar.activation(yt[:, off:off + nt], py, copy_fn,
                                         scale=sB)
                    off += nt
                nc.scalar.dma_start(out=y_dst[m], in_=yt[:])

        sgu_srcs = [[sguh_d[mp] for mp in range(_SIC)],
                    [sgul_d[mp] for mp in range(_SIC)]]
        sdn_srcs = [[sdnh_d[m] for m in range(_HHC)],
                    [sdnl_d[m] for m in range(_HHC)]]

        # routed slots
        off = 0
        for s in range(len(caps) if include_routed else 0):
            cap = caps[s]
            gu_srcs = [[wguh_d[s, mp] for mp in range(_IC)],
                       [wgul_d[s, mp] for mp in range(_IC)]]
            dn_srcs = [[wdnh_d[s, m] for m in range(_HHC)],
                       [wdnl_d[s, m] for m in range(_HHC)]]
            for cw in _chunk_sizes(cap):
                o = off
                mlp_chunk(
                    gu_srcs, dn_srcs,
                    [xgh_d[:, :, o:o + cw], xgl_d[:, :, o:o + cw]],
                    [y_d[m][:, o:o + cw] for m in range(_HHC)],
                    cw, n_pairs=_IC, n_kA=_HHC, sA=sA_r, sB=sB_r,
                )
                off += cw
        if include_shared:
            mlp_chunk(
                sgu_srcs, sdn_srcs,
                [xtsh_d[:], xtsl_d[:]],
                [ys_d[m][:] for m in range(_HHC)],
                S_TOK, n_pairs=_SIC, n_kA=_HHC, sA=sA_s, sB=sB_s,
            )

    _split_wide_waits(nc)
    return nc


# ------------------------------------------------------- walrus wait-limit fix
def _split_wide_waits(nc):
    """walrus codegen allows only 1 sync wait on fused matmuls (and few on
    ctrl ops). Hoist extra waits into single-wait NoOps on the same engine."""
    n = 0
    for f in nc.m.functions:
        for bb in f.blocks:
            il = bb.instructions
            i = 0
            while i < len(il):
                inst = il[i]
                si = inst.sync_info
                waits = list(si.on_wait) if si and si.on_wait else []
                cap = 1
                if len(waits) > cap:
                    inst.sync_info = mybir.SyncInfo(
                        on_wait=waits[:cap], on_update=list(si.on_update or []))
                    nops = [
                        mybir.InstNoOp(
                            name=nc.get_next_instruction_name(),
                            sync_info=mybir.SyncInfo(on_wait=[w], on_update=[]),
                            bass_nofuse=True,
                            engine=inst.engine,
                        )
                        for w in waits[cap:]
                    ]
                    il[i:i] = nops
                    i += len(nops)
                    n += len(nops)
                i += 1
    return n


# ------------------------------------------------------------------- assembly
def plan(topk_w, topk_ids):
    """Work assignment: split big experts into virtual pieces (<= SPLIT_Q
    tokens), sort pieces by size, piece of rank r -> core r % 8, slot r // 8.
    Slot capacities are the per-slot maxima; slots are ordered smallest-first.
    Returns (caps, assign, tok_of) where assign[s][c] = (expert, start, n)."""
    counts = np.bincount(topk_ids.ravel(), minlength=E)
    tok_of = [np.nonzero(topk_ids == e) for e in range(E)]
    live = [e for e in range(E) if counts[e] > 0]
    cs = [int(counts[e]) for e in live]

    def split_sizes(c, k):
        return [c // k + (1 if i < c % k else 0) for i in range(k)]

    def caps_of(ks):
        sizes = sorted((s for c, k in zip(cs, ks) for s in split_sizes(c, k)),
                       reverse=True)
        ns = -(-len(sizes) // N_CORES)
        return [max(64, _roundup(sizes[N_CORES * s], 8)) for s in range(ns)]

    best_ks, best_cost = None, None
    for n_slots in range(-(-len(cs) // N_CORES), 11):
        budget = n_slots * N_CORES
        if budget < len(cs):
            continue
        # greedy: split the expert with the largest current piece
        ks = [1] * len(cs)
        while sum(ks) < budget:
            i = max(range(len(cs)), key=lambda a: -(-cs[a] // ks[a]))
            if -(-cs[i] // ks[i]) <= 64:
                break
            ks[i] += 1
        # local search: move splits between experts while it helps
        def cost(ks):
            cp = caps_of(ks)
            return sum(cp) + 40 * len(cp)
        cur = cost(ks)
        improved = True
        while improved:
            improved = False
            for a in range(len(cs)):
                for b in range(len(cs)):
                    if b == a or ks[b] < 2:
                        continue
                    ks[a] += 1
                    ks[b] -= 1
                    v = cost(ks)
                    if v < cur:
                        cur, improved = v, True
                        break
                    ks[a] -= 1
                    ks[b] += 1
        if any(-(-c // k) > MAX_CHUNK for c, k in zip(cs, ks)):
            # keep every piece within one chunk for SBUF sizing
            pass
        if best_cost is None or cur < best_cost:
            best_cost, best_ks = cur, list(ks)

    ks = best_ks
    pieces = []
    for e, c, k in zip(live, cs, ks):
        st = 0
        for n in split_sizes(c, k):
            pieces.append((e, st, n))
            st += n
    pieces.sort(key=lambda p: -p[2])
    n_slots = -(-len(pieces) // N_CORES)
    pieces += [(0, 0, 0)] * (n_slots * N_CORES - len(pieces))
    slots = [pieces[N_CORES * s:N_CORES * (s + 1)] for s in range(n_slots)]
    slots.sort(key=lambda sl: sl[0][2])  # ascending cap
    caps = [max(64, _roundup(sl[0][2], 8)) for sl in slots]
    return caps, slots, tok_of


def _q8(a):
    """fp32 -> (hi, lo) e4m3 (value ~= hi + lo)."""
    hi = a.astype(E4)
    lo = (a - hi.astype(np.float32)).astype(E4)
    return hi, lo


def _pack_gu(w8):
    """[H, 2I'] e4m3 (gate cols | up cols) -> [IP, 128, KH, 256]."""
    h, twoi = w8.shape
    ip = twoi // 256
    kh = h // 128
    g = w8[:, :ip * 128].reshape(kh, 128, ip, 128).transpose(2, 1, 0, 3)
    u = w8[:, ip * 128:].reshape(kh, 128, ip, 128).transpose(2, 1, 0, 3)
    out = np.empty((ip, 128, kh, 256), E4)
    out[..., 0:128] = g
    out[..., 128:256] = u
    return out


def _pack_dn(d8):
    """[I', H] e4m3 -> [MH, 128, IC, 128]."""
    i_, h = d8.shape
    ic = i_ // 128
    mh = h // 128
    return np.ascontiguousarray(
        d8.reshape(ic, 128, mh, 128).transpose(2, 1, 0, 3))


def _pack_x(x8):
    """[H, n] e4m3 -> [128, KH, n]."""
    h, n = x8.shape
    return np.ascontiguousarray(x8.reshape(h // 128, 128, n).transpose(1, 0, 2))


def kernel(hidden_states, gate_w, gate_bias, w_gate_up, w_down,
           shared_gate_up, shared_down):
    hs = np.ascontiguousarray(hidden_states, dtype=np.float32)
    topk_w, topk_ids = _grouped_topk_host(hs, gate_w, gate_bias)
    caps, slots, tok_of = plan(topk_w, topk_ids)
    n_slots = len(caps)
    CT = sum(caps)
    offs = np.concatenate([[0], np.cumsum(caps)])[:n_slots]

    w_gate_up = np.asarray(w_gate_up, dtype=np.float32)
    w_down = np.asarray(w_down, dtype=np.float32)
    shared_gate_up = np.asarray(shared_gate_up, dtype=np.float32)
    shared_down = np.asarray(shared_down, dtype=np.float32)

    # global e4m3 scales
    sX = float(np.abs(hs).max()) / QMAX
    sWgu = float(np.abs(w_gate_up).max()) / QMAX
    sWdn = float(np.abs(w_down).max()) / QMAX
    sSgu = float(np.abs(shared_gate_up).max()) / QMAX
    sSdn = float(np.abs(shared_down).max()) / QMAX

    xT = np.ascontiguousarray(hs.T) / sX          # [H, T]
    xTh, xTl = _q8(xT)

    # per-expert packed weights (hi/lo)
    wgu_packed = []
    wdn_packed = []
    for e in range(E):
        wh, wl = _q8(w_gate_up[e] / sWgu)
        wgu_packed.append((_pack_gu(wh), _pack_gu(wl)))
        dh, dl = _q8(w_down[e] / sWdn)
        wdn_packed.append((_pack_dn(dh), _pack_dn(dl)))

    # shared tensors: per TP-slice of the intermediate dim
    SGU_tp, SDN_tp = [], []
    for tp in range(SHARED_TP):
        base = tp * S_SI
        sgu = np.concatenate(
            [shared_gate_up[:, base:base + S_SI],
             shared_gate_up[:, SI + base:SI + base + S_SI]], axis=1) / sSgu
        sh, sl_ = _q8(sgu)
        sdn = shared_down[base:base + S_SI, :] / sSdn
        dh, dl = _q8(sdn)
        SGU_tp.append((_pack_gu(sh), _pack_gu(sl_)))
        SDN_tp.append((_pack_dn(dh), _pack_dn(dl)))
    XTS_dp = [
        (np.ascontiguousarray(xTh[:, dp * S_TOK:(dp + 1) * S_TOK]),
         np.ascontiguousarray(xTl[:, dp * S_TOK:(dp + 1) * S_TOK]))
        for dp in range(SHARED_DP)
    ]
    XTS_dp = [(_pack_x(a), _pack_x(b)) for a, b in XTS_dp]

    in_maps = []
    for c in range(N_CORES):
        XGH = np.zeros((128, _HHC, CT), E4)
        XGL = np.zeros((128, _HHC, CT), E4)
        WGUH = np.zeros((n_slots, _IC, 128, _HHC, 256), E4)
        WGUL = np.zeros((n_slots, _IC, 128, _HHC, 256), E4)
        WDNH = np.zeros((n_slots, _HHC, 128, _IC, 128), E4)
        WDNL = np.zeros((n_slots, _HHC, 128, _IC, 128), E4)
        for s in range(n_slots):
            e, st, n = slots[s][c]
            if n == 0:
                continue
            idx = tok_of[e][0][st:st + n]
            xh = _pack_x(xTh[:, idx])
            xl = _pack_x(xTl[:, idx])
            XGH[:, :, offs[s]:offs[s] + n] = xh
            XGL[:, :, offs[s]:offs[s] + n] = xl
            WGUH[s], WGUL[s] = wgu_packed[e]
            WDNH[s], WDNL[s] = wdn_packed[e]
        tp, dp = c // SHARED_DP, c % SHARED_DP
        in_maps.append({
            "xgh": XGH, "xgl": XGL,
            "wguh": WGUH, "wgul": WGUL, "wdnh": WDNH, "wdnl": WDNL,
            "sguh": SGU_tp[tp][0], "sgul": SGU_tp[tp][1],
            "sdnh": SDN_tp[tp][0], "sdnl": SDN_tp[tp][1],
            "xtsh": XTS_dp[dp][0], "xtsl": XTS_dp[dp][1],
        })

    sA_r = sWgu * sX
    sB_r = sWdn / CH
    sA_s = sSgu * sX
    sB_s = sSdn / CH
    nc = _build_nc(caps, sA_r, sB_r, sA_s, sB_s)
    res = run_bass_kernel_spmd(nc, in_maps, list(range(N_CORES)))

    out = np.zeros((T, H), np.float32)
    for c in range(N_CORES):
        dp = c % SHARED_DP
        ys = res.results[c]["ys"].astype(np.float32)  # [16, 128, S_TOK]
        out[dp * S_TOK:(dp + 1) * S_TOK] += ys.reshape(H, S_TOK).T
    for c in range(N_CORES):
        y = res.results[c]["y"].astype(np.float32).reshape(H, CT)
        for s in range(n_slots):
            e, st, n = slots[s][c]
            if n == 0:
                continue
            idx = tok_of[e][0][st:st + n]
            kpos = tok_of[e][1][st:st + n]
            wts = topk_w[idx, kpos].astype(np.float32) * ROUTED_SCALING
            out[idx] += wts[:, None] * y[:, offs[s]:offs[s] + n].T
    return out


# revision 22
# speedup vs baseline: 1.4050x; 1.3987x over previous
"""DeepSeek-style MoE (32 routed experts, top-8, grouped routing, 2 shared experts)
on 8 Trainium2 NeuronCores via Bass/Tile.

Strategy (expert-parallel, fp8 DoubleRow matmuls with hi/lo error compensation):
- Host computes the routing (sigmoid gate + grouped top-k, bit-matching the
  reference via jax-on-CPU) and gathers each expert's tokens.
- Experts are split into balanced virtual pieces; pieces sorted by size,
  piece of rank r goes to core r%8, slot r//8. All cores run the SAME
  program (slot capacities = per-slot max piece size), so the kernel is
  SPMD-static while the work stays balanced.
- All matmuls run as fp8e4 (e4m3) DoubleRow instructions (2 k-tiles of 128
  per instruction, 0.5 cycles/row = 4x the bf16 FLOP rate). Every operand
  O is decomposed on the host (or on-chip for h) into hi+lo e4m3 parts:
  O/s = Oh + Ol with Ol the quantized residual, so O is represented to
  ~6e-4 relative. A matmul W@x is computed with three terms
      Wh@xh + Wl@xh + Wh@xl      (dropping the ~7e-7 Wl@xl term)
  at 0.75x the bf16-equivalent PE time. k-chunks are paired into the two
  DoubleRow k-tiles, so all operands keep their natural chunk-major layout.
- Per expert-slot MLP on transposed activations:
      gu^T = w_gate_up^T @ x^T     (3-term DR into PSUM, 8 chunk-pairs)
      t    = silu(g * sA)          (ACT, f32)
      hb   = (u * sA*CH) * t       (DVE scalar_tensor_tensor, f32, h*CH)
      hh   = fp8(hb)               (Pool copy)
      hl   = fp8(hb - hh)          (DVE scalar_tensor_tensor)
      y^T  = w_down^T @ h          (3-term DR, PSUM -> ACT copy*sWd/CH -> bf16)
- The shared MLP runs 2-way tensor-parallel (intermediate dim) x 4-way
  data-parallel (tokens), same scheme.
- Host combines in fp32: out = shared partials + 2.5 * topk_w * y rows.

Self-contained: only numpy/jax/ml_dtypes/concourse imports, shapes hardcoded.
"""
import numpy as np
import ml_dtypes

import concourse.bass as bass
import concourse.mybir as mybir
import concourse.tile as tile
from concourse.bass_utils import run_bass_kernel_spmd

F32 = mybir.dt.float32
BF16 = mybir.dt.bfloat16
F8 = mybir.dt.float8e4
E4 = ml_dtypes.float8_e4m3
BF = ml_dtypes.bfloat16
DR = mybir.MatmulPerfMode.DoubleRow
MULT = mybir.AluOpType.mult
ADD = mybir.AluOpType.add

T, H, E, I = 4096, 2048, 32, 1024
TOP_K, N_GROUP, TOPK_GROUP = 8, 8, 4
SI = 2048
ROUTED_SCALING = 2.5
N_CORES = 8
SPLIT_Q = 1152   # experts with more tokens are split into virtual experts
MAX_CHUNK = 1152
# shared expert: SHARED_TP-way split of the intermediate dim x SHARED_DP-way
# split of the tokens (SHARED_TP * SHARED_DP == N_CORES)
SHARED_TP = 2
SHARED_DP = 4
S_TOK = T // SHARED_DP      # tokens per core for the shared MLP
S_SI = SI // SHARED_TP      # intermediate slice per core

_HHC = H // 128   # 16 k-chunks over H
_IC = I // 128    # 8 pairs over I (also h k-chunks)
_SIC = S_SI // 128  # 8 shared pairs

QMAX = 224.0      # e4m3 target max (true max 240; margin for rounding)
CH = 8.0          # h -> fp8 scale (h*CH must stay below ~240)


# ---------------------------------------------------------------- host routing
def _grouped_topk_host(hidden_states, gate_w, gate_bias):
    """Bit-match the reference's jax fp32 routing, on the CPU backend."""
    import jax

    try:
        jax.config.update("jax_platforms", "axon,cpu")
    except Exception:
        pass
    import jax.numpy as jnp

    cpu = jax.devices("cpu")[0]
    with jax.default_device(cpu):
        hs = jnp.asarray(hidden_states)
        gw = jnp.asarray(gate_w)
        bias = jnp.asarray(gate_bias)
        router_logits = hs @ gw
        scores = jax.nn.sigmoid(router_logits)
        sc = scores + bias[None, :]
        t = sc.shape[0]
        g = sc.reshape(t, N_GROUP, E // N_GROUP)
        group_scores = jax.lax.top_k(g, 2)[0].sum(-1)
        grp_idx = jax.lax.top_k(group_scores, TOPK_GROUP)[1]
        grp_mask = jnp.zeros((t, N_GROUP), sc.dtype).at[
            jnp.arange(t)[:, None], grp_idx].set(1.0)
        tok_mask = jnp.repeat(grp_mask, E // N_GROUP, axis=1)
        masked = jnp.where(tok_mask > 0, sc, -jnp.inf)
        topk_ids = jax.lax.top_k(masked, TOP_K)[1]
        w = jnp.take_along_axis(scores, topk_ids, axis=1)
        w = w / (w.sum(-1, keepdims=True) + 1e-20)
        return np.asarray(w), np.asarray(topk_ids)


def _roundup(x, m):
    return -(-x // m) * m


def _chunk_sizes(cap):
    """Split cap (multiple of 8) into chunks <= MAX_CHUNK, multiple of 8."""
    out = []
    rem = cap
    while rem > MAX_CHUNK:
        take = MAX_CHUNK
        out.append(take)
        rem -= take
    out.append(rem)
    return out


def _n_tiles(chunk):
    """Split chunk into matmul N-tiles <= 512 (DR moving dim = 2*nt <= 1024)."""
    out = []
    rem = chunk
    while rem > 512:
        out.append(512)
        rem -= 512
    out.append(rem)
    return out


# ---------------------------------------------------------------- bass program
def _build_nc(caps, sA_r=1.0, sB_r=1.0, sA_s=1.0, sB_s=1.0,
              include_routed=True, include_shared=True):
    """sA_r: silu input scale for routed (sWgu*sX); sB_r: y scale (sWdn/CH);
    sA_s/sB_s: same for the shared MLP."""
    nc = bass.Bass()
    CT = sum(caps)
    n_slots = len(caps)

    xgh_d = nc.dram_tensor("xgh", [128, _HHC, CT], F8, kind="ExternalInput")
    xgl_d = nc.dram_tensor("xgl", [128, _HHC, CT], F8, kind="ExternalInput")
    # wgu: per (pair, partition, k-chunk): [hi gate|hi up|lo gate|lo up] x 128
    wgu_d = nc.dram_tensor("wgu", [n_slots, _IC, 128, _HHC, 512], F8, kind="ExternalInput")
    # wdn: per m-tile: [hi|lo] x [128, IC, 128]
    wdn_d = nc.dram_tensor("wdn", [n_slots, _HHC, 128, 2, _IC, 128], F8, kind="ExternalInput")
    sgu_d = nc.dram_tensor("sgu", [_SIC, 128, _HHC, 512], F8, kind="ExternalInput")
    sdn_d = nc.dram_tensor("sdn", [_HHC, 128, 2, _SIC, 128], F8, kind="ExternalInput")
    xtsh_d = nc.dram_tensor("xtsh", [128, _HHC, S_TOK], F8, kind="ExternalInput")
    xtsl_d = nc.dram_tensor("xtsl", [128, _HHC, S_TOK], F8, kind="ExternalInput")
    y_d = nc.dram_tensor("y", [_HHC, 128, CT], BF16, kind="ExternalOutput")
    ys_d = nc.dram_tensor("ys", [_HHC, 128, S_TOK], BF16, kind="ExternalOutput")

    silu = mybir.ActivationFunctionType.Silu
    copy_fn = mybir.ActivationFunctionType.Copy

    from contextlib import ExitStack

    with tile.TileContext(nc) as tc, ExitStack() as ctx:
        xg_pool = ctx.enter_context(tc.tile_pool(name="xgp", bufs=2))
        w_pool = ctx.enter_context(tc.tile_pool(name="wp", bufs=4))
        dn_pool = ctx.enter_context(tc.tile_pool(name="dnp", bufs=4))
        h_pool = ctx.enter_context(tc.tile_pool(name="hp", bufs=2))
        t_pool = ctx.enter_context(tc.tile_pool(name="tp", bufs=2))
        y_pool = ctx.enter_context(tc.tile_pool(name="yp", bufs=4))
        psA = ctx.enter_context(tc.tile_pool(name="psA", bufs=3, space="PSUM"))
        psB = ctx.enter_context(tc.tile_pool(name="psB", bufs=2, space="PSUM"))

        def load_inputs(ch, fast_start=False):
            """Issue chunk input DMAs (wgu pair 0 + x hi/lo) on the SP queue."""
            gu_srcs, _, x_srcs, _, cw, n_pairs, n_kA, _, _ = ch
            wp0 = w_pool.tile([128, n_kA, 512], F8, tag="w", name="wpt")
            xh = xg_pool.tile([128, n_kA, cw], F8, tag="xh", name="xht")
            xl = xg_pool.tile([128, n_kA, cw], F8, tag="xl", name="xlt")
            if fast_start:
                # issue only the first quarter of wgu pair 0 and the first xh
                # chunks first, so the first matmul starts ASAP
                qk = n_kA // 4
                nc.sync.dma_start(out=wp0[:, :qk], in_=gu_srcs[0][:, :qk])
                nc.sync.dma_start(out=xh[:, :qk], in_=x_srcs[0][:, :qk])
                for q in range(1, 4):
                    s = slice(q * qk, (q + 1) * qk)
                    nc.sync.dma_start(out=wp0[:, s], in_=gu_srcs[0][:, s])
                nc.sync.dma_start(out=xh[:, qk:], in_=x_srcs[0][:, qk:])
                nc.sync.dma_start(out=xl[:], in_=x_srcs[1][:])
            else:
                # small pieces so y stores of the running chunk interleave on
                # the serial DMA fabric
                qk = n_kA // 4
                nc.sync.dma_start(out=wp0[:, :qk * 2], in_=gu_srcs[0][:, :qk * 2])
                nc.sync.dma_start(out=xh[:, :qk], in_=x_srcs[0][:, :qk])
                nc.sync.dma_start(out=wp0[:, qk * 2:], in_=gu_srcs[0][:, qk * 2:])
                for q in range(1, 4):
                    s = slice(q * qk, (q + 1) * qk)
                    nc.sync.dma_start(out=xh[:, s], in_=x_srcs[0][:, s])
                for q in range(4):
                    s = slice(q * qk, (q + 1) * qk)
                    nc.sync.dma_start(out=xl[:, s], in_=x_srcs[1][:, s])
            return wp0, xh, xl

        def mlp_chunk(ch, loaded, next_ch):
            """One token-chunk of one expert MLP (all fp8 hi/lo DoubleRow).

            ch = (gu_srcs, dn_srcs, x_srcs, y_dst, cw, n_pairs, n_kA, sA, sB)
            gu_srcs[mp]: DRAM AP [128, n_kA, 512] (hi g|hi u|lo g|lo u of pair)
            dn_srcs[m]:  DRAM AP [128, 2, n_pairs, 128] (hi|lo)
            x_srcs[hl]:  DRAM AP [128, n_kA, cw]
            y_dst[m]:    DRAM AP [128, cw] (bf16)
            loaded: (wp0, xh, xl) tiles prefetched by the previous chunk.
            Returns the prefetched tiles for next_ch (or None).
            """
            gu_srcs, dn_srcs, x_srcs, y_dst, cw, n_pairs, n_kA, sA, sB = ch
            tiles = _n_tiles(cw)
            ncpA = n_kA // 2
            ncpB = n_pairs // 2

            def wp_load(mp):
                wp = w_pool.tile([128, n_kA, 512], F8, tag="w", name="wpt")
                qk = n_kA // 4
                for q in range(4):
                    s = slice(q * qk, (q + 1) * qk)
                    nc.sync.dma_start(out=wp[:, s], in_=gu_srcs[mp][:, s])
                return wp

            wp_next, xh, xl = loaded

            # this chunk's first two dn tiles stream during phase A
            def dn_load(m):
                dt_ = dn_pool.tile([128, 2, n_pairs, 128], F8, tag="dn", name="dnt")
                nc.scalar.dma_start(out=dt_[:], in_=dn_srcs[m])
                return dt_
            dn_q = [dn_load(0), dn_load(1)]

            hh = h_pool.tile([128, n_pairs, cw], F8, tag="hh", name="hht")
            hl = h_pool.tile([128, n_pairs, cw], F8, tag="hl", name="hlt")
            for mp in range(n_pairs):
                wp = wp_next
                if mp + 1 < n_pairs:
                    wp_next = wp_load(mp + 1)
                off = 0
                for nt in tiles:
                    g = psA.tile([128, 512], F32, tag="g", name="gps")[:, :nt]
                    u = psA.tile([128, 512], F32, tag="u", name="ups")[:, :nt]
                    # xh terms first, xl terms after: the xl tile may still
                    # be streaming when the group starts
                    for cp in range(ncpA):
                        sl = slice(2 * cp, 2 * cp + 2)
                        xhs = xh[:, sl, off:off + nt]
                        st = cp == 0
                        for dst, c0 in ((g, 0), (u, 128)):
                            nc.tensor.matmul(dst, wp[:, sl, c0:c0 + 128], xhs,
                                             start=st, stop=False, perf_mode=DR)
                            nc.tensor.matmul(dst, wp[:, sl, c0 + 256:c0 + 384],
                                             xhs, start=False, stop=False,
                                             perf_mode=DR)
                    for cp in range(ncpA):
                        sl = slice(2 * cp, 2 * cp + 2)
                        xls = xl[:, sl, off:off + nt]
                        sp = cp == ncpA - 1
                        for dst, c0 in ((g, 0), (u, 128)):
                            nc.tensor.matmul(dst, wp[:, sl, c0:c0 + 128], xls,
                                             start=False, stop=sp,
                                             perf_mode=DR)
                    t = t_pool.tile([128, 512], F32, tag="t", name="tt")[:, :nt]
                    hb = t_pool.tile([128, 512], F32, tag="hb", name="hbt")[:, :nt]
                    nc.scalar.activation(t, g, silu, scale=sA)
                    # hb = (u * sA*CH) * t = h * CH
                    nc.vector.scalar_tensor_tensor(hb, u, sA * CH, t, MULT, MULT)
                    hhs = hh[:, mp, off:off + nt]
                    hls = hl[:, mp, off:off + nt]
                    nc.gpsimd.tensor_copy(hhs, hb)
                    # hl = hb - hh (quantized residual)
                    nc.vector.scalar_tensor_tensor(hls, hhs, -1.0, hb, MULT, ADD)
                    off += nt
            # prefetch the next chunk's inputs now: they stream on the SP
            # queue while this chunk's phase B computes
            nxt_loaded = load_inputs(next_ch) if next_ch is not None else None
            # phase B weights and outputs ride the ACT HWDGE queue; dn loads
            # are emitted two m-tiles ahead so y stores never block them
            for m in range(_HHC):
                dt_ = dn_q.pop(0)
                dh, dl = dt_[:, 0], dt_[:, 1]
                if m + 2 < _HHC:
                    dn_q.append(dn_load(m + 2))
                yt = y_pool.tile([128, cw], BF16, tag="y", name="yt")
                off = 0
                for nt in tiles:
                    py = psB.tile([128, 512], F32, tag="py", name="pyps")[:, :nt]
                    for cp in range(ncpB):
                        sl = slice(2 * cp, 2 * cp + 2)
                        hhs = hh[:, sl, off:off + nt]
                        hls = hl[:, sl, off:off + nt]
                        nc.tensor.matmul(py, dh[:, sl, :], hhs, start=(cp == 0),
                                         stop=False, perf_mode=DR)
                        nc.tensor.matmul(py, dl[:, sl, :], hhs, start=False,
                                         stop=False, perf_mode=DR)
                        nc.tensor.matmul(py, dh[:, sl, :], hls, start=False,
                                         stop=(cp == ncpB - 1), perf_mode=DR)
                    nc.scalar.activation(yt[:, off:off + nt], py, copy_fn,
                                         scale=sB)
                    off += nt
                nc.scalar.dma_start(out=y_dst[m], in_=yt[:])
            return nxt_loaded

        # chunk list: routed slots then the shared MLP chunk
        chunks = []
        off = 0
        for s in range(len(caps) if include_routed else 0):
            gu_srcs = [wgu_d[s, mp] for mp in range(_IC)]
            dn_srcs = [wdn_d[s, m] for m in range(_HHC)]
            for cw in _chunk_sizes(caps[s]):
                o = off
                chunks.append((
                    gu_srcs, dn_srcs,
                    [xgh_d[:, :, o:o + cw], xgl_d[:, :, o:o + cw]],
                    [y_d[m][:, o:o + cw] for m in range(_HHC)],
                    cw, _IC, _HHC, sA_r, sB_r,
                ))
                off += cw
        if include_shared:
            chunks.append((
                [sgu_d[mp] for mp in range(_SIC)],
                [sdn_d[m] for m in range(_HHC)],
                [xtsh_d[:], xtsl_d[:]],
                [ys_d[m][:] for m in range(_HHC)],
                S_TOK, _SIC, _HHC, sA_s, sB_s,
            ))

        loaded = load_inputs(chunks[0], fast_start=True)
        for i, ch in enumerate(chunks):
            nxt = chunks[i + 1] if i + 1 < len(chunks) else None
            loaded = mlp_chunk(ch, loaded, nxt)

    _split_wide_waits(nc)
    return nc


# ------------------------------------------------------- walrus wait-limit fix
def _split_wide_waits(nc):
    """walrus codegen allows only 1 sync wait on fused matmuls (and few on
    ctrl ops). Hoist extra waits into single-wait NoOps on the same engine."""
    n = 0
    for f in nc.m.functions:
        for bb in f.blocks:
            il = bb.instructions
            i = 0
            while i < len(il):
                inst = il[i]
                si = inst.sync_info
                waits = list(si.on_wait) if si and si.on_wait else []
                cap = 1
                if len(waits) > cap:
                    inst.sync_info = mybir.SyncInfo(
                        on_wait=waits[:cap], on_update=list(si.on_update or []))
                    nops = [
                        mybir.InstNoOp(
                            name=nc.get_next_instruction_name(),
                            sync_info=mybir.SyncInfo(on_wait=[w], on_update=[]),
                            bass_nofuse=True,
                            engine=inst.engine,
                        )
                        for w in waits[cap:]
                    ]
                    il[i:i] = nops
                    i += len(nops)
                    n += len(nops)
                i += 1
    return n


# ------------------------------------------------------------------- assembly
def plan(topk_w, topk_ids, force_slots=7):
    """Work assignment: split big experts into virtual pieces (<= SPLIT_Q
    tokens), sort pieces by size, piece of rank r -> core r % 8, slot r // 8.
    Slot capacities are the per-slot maxima; slots are ordered smallest-first.
    Returns (caps, assign, tok_of) where assign[s][c] = (expert, start, n)."""
    counts = np.bincount(topk_ids.ravel(), minlength=E)
    tok_of = [np.nonzero(topk_ids == e) for e in range(E)]
    live = [e for e in range(E) if counts[e] > 0]
    cs = [int(counts[e]) for e in live]

    def split_sizes(c, k):
        return [c // k + (1 if i < c % k else 0) for i in range(k)]

    def caps_of(ks):
        sizes = sorted((s for c, k in zip(cs, ks) for s in split_sizes(c, k)),
                       reverse=True)
        ns = -(-len(sizes) // N_CORES)
        return [max(64, _roundup(sizes[N_CORES * s], 8)) for s in range(ns)]

    best_ks, best_cost = None, None
    # 7 slots measured fastest in TimelineSim for this regime (sum of caps
    # 4376 vs 4096 ideal, junctions fully hidden); fall back to a search if
    # it can't hold the live experts.
    if force_slots and force_slots * N_CORES < len(cs):
        force_slots = None
    slot_range = ([force_slots] if force_slots
                  else range(-(-len(cs) // N_CORES), 11))
    for n_slots in slot_range:
        budget = n_slots * N_CORES
        if budget < len(cs):
            continue
        # greedy: split the expert with the largest current piece
        ks = [1] * len(cs)
        while sum(ks) < budget:
            i = max(range(len(cs)), key=lambda a: -(-cs[a] // ks[a]))
            if -(-cs[i] // ks[i]) <= 64:
                break
            ks[i] += 1
        # local search: move splits between experts while it helps.
        # Cost: PE time ~ sum(caps); each slot re-streams full expert weights
        # (junction risk) and caps < 512 pay the 2x small-element DMA penalty.
        def cost(ks):
            cp = caps_of(ks)
            return (sum(cp) + 96 * len(cp)
                    + sum(96 for c in cp if c < 512))
        cur = cost(ks)
        improved = True
        while improved:
            improved = False
            for a in range(len(cs)):
                for b in range(len(cs)):
                    if b == a or ks[b] < 2:
                        continue
                    ks[a] += 1
                    ks[b] -= 1
                    v = cost(ks)
                    if v < cur:
                        cur, improved = v, True
                        break
                    ks[a] -= 1
                    ks[b] += 1
        if any(-(-c // k) > MAX_CHUNK for c, k in zip(cs, ks)):
            # keep every piece within one chunk for SBUF sizing
            pass
        if best_cost is None or cur < best_cost:
            best_cost, best_ks = cur, list(ks)

    ks = best_ks
    pieces = []
    for e, c, k in zip(live, cs, ks):
        st = 0
        for n in split_sizes(c, k):
            pieces.append((e, st, n))
            st += n
    pieces.sort(key=lambda p: -p[2])
    n_slots = -(-len(pieces) // N_CORES)
    pieces += [(0, 0, 0)] * (n_slots * N_CORES - len(pieces))
    slots = [pieces[N_CORES * s:N_CORES * (s + 1)] for s in range(n_slots)]
    slots.sort(key=lambda sl: sl[0][2])  # ascending cap
    caps = [max(64, _roundup(sl[0][2], 8)) for sl in slots]
    return caps, slots, tok_of


def _q8(a):
    """fp32 -> (hi, lo) e4m3 (value ~= hi + lo)."""
    hi = a.astype(E4)
    lo = (a - hi.astype(np.float32)).astype(E4)
    return hi, lo


def _pack_gu(wh8, wl8):
    """[H, 2I'] e4m3 hi+lo (gate cols | up cols) -> [IP, 128, KH, 512]."""
    h, twoi = wh8.shape
    ip = twoi // 256
    kh = h // 128
    out = np.empty((ip, 128, kh, 512), E4)
    for part, w8 in ((0, wh8), (256, wl8)):
        g = w8[:, :ip * 128].reshape(kh, 128, ip, 128).transpose(2, 1, 0, 3)
        u = w8[:, ip * 128:].reshape(kh, 128, ip, 128).transpose(2, 1, 0, 3)
        out[..., part:part + 128] = g
        out[..., part + 128:part + 256] = u
    return out


def _pack_dn(dh8, dl8):
    """[I', H] e4m3 hi+lo -> [MH, 128, 2, IC, 128]."""
    i_, h = dh8.shape
    ic = i_ // 128
    mh = h // 128
    out = np.empty((mh, 128, 2, ic, 128), E4)
    out[:, :, 0] = dh8.reshape(ic, 128, mh, 128).transpose(2, 1, 0, 3)
    out[:, :, 1] = dl8.reshape(ic, 128, mh, 128).transpose(2, 1, 0, 3)
    return out


def _pack_x(x8):
    """[H, n] e4m3 -> [128, KH, n]."""
    h, n = x8.shape
    return np.ascontiguousarray(x8.reshape(h // 128, 128, n).transpose(1, 0, 2))


def kernel(hidden_states, gate_w, gate_bias, w_gate_up, w_down,
           shared_gate_up, shared_down):
    hs = np.ascontiguousarray(hidden_states, dtype=np.float32)
    topk_w, topk_ids = _grouped_topk_host(hs, gate_w, gate_bias)
    caps, slots, tok_of = plan(topk_w, topk_ids)
    n_slots = len(caps)
    CT = sum(caps)
    offs = np.concatenate([[0], np.cumsum(caps)])[:n_slots]

    w_gate_up = np.asarray(w_gate_up, dtype=np.float32)
    w_down = np.asarray(w_down, dtype=np.float32)
    shared_gate_up = np.asarray(shared_gate_up, dtype=np.float32)
    shared_down = np.asarray(shared_down, dtype=np.float32)

    # global e4m3 scales
    sX = float(np.abs(hs).max()) / QMAX
    sWgu = float(np.abs(w_gate_up).max()) / QMAX
    sWdn = float(np.abs(w_down).max()) / QMAX
    sSgu = float(np.abs(shared_gate_up).max()) / QMAX
    sSdn = float(np.abs(shared_down).max()) / QMAX

    xT = np.ascontiguousarray(hs.T) / sX          # [H, T]
    xTh, xTl = _q8(xT)

    # per-expert packed weights (hi/lo merged)
    wgu_packed = []
    wdn_packed = []
    for e in range(E):
        wh, wl = _q8(w_gate_up[e] / sWgu)
        wgu_packed.append(_pack_gu(wh, wl))
        dh, dl = _q8(w_down[e] / sWdn)
        wdn_packed.append(_pack_dn(dh, dl))

    # shared tensors: per TP-slice of the intermediate dim
    SGU_tp, SDN_tp = [], []
    for tp in range(SHARED_TP):
        base = tp * S_SI
        sgu = np.concatenate(
            [shared_gate_up[:, base:base + S_SI],
             shared_gate_up[:, SI + base:SI + base + S_SI]], axis=1) / sSgu
        sh, sl_ = _q8(sgu)
        sdn = shared_down[base:base + S_SI, :] / sSdn
        dh, dl = _q8(sdn)
        SGU_tp.append(_pack_gu(sh, sl_))
        SDN_tp.append(_pack_dn(dh, dl))
    XTS_dp = [
        (np.ascontiguousarray(xTh[:, dp * S_TOK:(dp + 1) * S_TOK]),
         np.ascontiguousarray(xTl[:, dp * S_TOK:(dp + 1) * S_TOK]))
        for dp in range(SHARED_DP)
    ]
    XTS_dp = [(_pack_x(a), _pack_x(b)) for a, b in XTS_dp]

    in_maps = []
    for c in range(N_CORES):
        XGH = np.zeros((128, _HHC, CT), E4)
        XGL = np.zeros((128, _HHC, CT), E4)
        WGU = np.zeros((n_slots, _IC, 128, _HHC, 512), E4)
        WDN = np.zeros((n_slots, _HHC, 128, 2, _IC, 128), E4)
        for s in range(n_slots):
            e, st, n = slots[s][c]
            if n == 0:
                continue
            idx = tok_of[e][0][st:st + n]
            XGH[:, :, offs[s]:offs[s] + n] = _pack_x(xTh[:, idx])
            XGL[:, :, offs[s]:offs[s] + n] = _pack_x(xTl[:, idx])
            WGU[s] = wgu_packed[e]
            WDN[s] = wdn_packed[e]
        tp, dp = c // SHARED_DP, c % SHARED_DP
        in_maps.append({
            "xgh": XGH, "xgl": XGL, "wgu": WGU, "wdn": WDN,
            "sgu": SGU_tp[tp], "sdn": SDN_tp[tp],
            "xtsh": XTS_dp[dp][0], "xtsl": XTS_dp[dp][1],
        })

    sA_r = sWgu * sX
    sB_r = sWdn / CH
    sA_s = sSgu * sX
    sB_s = sSdn / CH
    nc = _build_nc(caps, sA_r, sB_r, sA_s, sB_s)
    res = run_bass_kernel_spmd(nc, in_maps, list(range(N_CORES)))

    out = np.zeros((T, H), np.float32)
    for c in range(N_CORES):
        dp = c % SHARED_DP
        ys = res.results[c]["ys"].astype(np.float32)  # [16, 128, S_TOK]
        out[dp * S_TOK:(dp + 1) * S_TOK] += ys.reshape(H, S_TOK).T
    for c in range(N_CORES):
        y = res.results[c]["y"].astype(np.float32).reshape(H, CT)
        for s in range(n_slots):
            e, st, n = slots[s][c]
            if n == 0:
                continue
            idx = tok_of[e][0][st:st + n]
            kpos = tok_of[e][1][st:st + n]
            wts = topk_w[idx, kpos].astype(np.float32) * ROUTED_SCALING
            out[idx] += wts[:, None] * y[:, offs[s]:offs[s] + n].T
    return out
